# revision 11
# baseline (speedup 1.0000x reference)
"""Trainium2 Bass kernel for the DGCNN-style message-passing block.

Math (per batch b, data-parallel over 8 cores):
    proj = x @ Wp^T
    m[i] = max_k proj[knn[i,k]]           (edge maxpool: max_k(f_j - f_i) = m_i - proj_i)
    x1 = BN_l((m - proj) @ Wl^T);  x2 = BN_g(x @ Wg^T)
    h  = BN_1((x1+x2) @ W1^T + b1); a = sigmoid(BN_2(h @ W2^T + b2))
    out = BN_f(a*x1 + (1-a)*x2)

All BNs are inference-affine and fold into the weights host-side, and proj
composes into the local branch, giving:
    f  = m@Wmf^T + x@Wxf^T + tf          (= x1+x2, feeds h)
    d' = m@Wmd^T + x@Wxd^T + td          (= s_f*(x1-x2))
    v  = x@Wxv^T + tv                    (= BN_f(x2))
    h  = f@W1'^T + t1;  a = sigmoid(h@W2'^T + t2)
    out = v + a*d'

Everything on chip is bf16 (weights, activations, gathered neighbors); PSUM
accumulates in f32.  The KNN max-gather runs per 512-node half-quarter as 8
1024-index `dma_gather` ops (the SWDGE descriptor ring caps a gather at 1024
indices), each covering a k-pair, spread over 4 SWDGE queues, sourced from a
bf16 node-major copy of proj in DRAM that is written out incrementally as
the proj matmuls complete.  A DVE max-accumulate chain reduces the 16
neighbor slices, m^T comes back via PE transposes (evacuated on the scalar
engine), and the five matmul passes run on half-width (512-node) PSUM tiles
with scalar-engine evacuation.  The pipeline is software-issued per
half-quarter (gathers+chain of chunk i before the passes of chunk i-1) so
the DVE chain tracks the gather stream, and the non-critical background
loads carry a scheduler wait-until hint so they cannot delay the gather
stream start.
"""

import numpy as np
import ml_dtypes

import concourse.bass as bass
import concourse.mybir as mybir
import concourse.tile as tile
from concourse import bacc
from concourse.bass_utils import run_bass_kernel_spmd
from concourse.masks import make_identity

F32 = mybir.dt.float32
BF16 = mybir.dt.bfloat16
I16 = mybir.dt.int16

B, N, K, C = 8, 4096, 16, 256
P = 128
NT = N // P          # 32 node stripes
CK = C // P          # 2 channel chunks
NQ = 4               # node quarters (xt granularity)
QN = N // NQ         # 1024 nodes per quarter
NH = 8               # pipeline chunks (half-quarters)
HN = N // NH         # 512 nodes per chunk
HT = HN // P         # 4 stripes per chunk
NG = K // 2          # 8 gathers per chunk (one per k-pair)
COLS_H = QN // 16    # 64 idx columns per 1024-idx gather
EPS = 1e-5

AF = mybir.ActivationFunctionType


def build_bass(n_cores: int = 8, reps: int = 1):
    nc = bacc.Bacc(
        "TRN2",
        target_bir_lowering=False,
        debug=False,
        enable_asserts=False,
        num_devices=n_cores,
        num_swdge_queues=4,
    )

    xT = nc.dram_tensor("xT", [C, N], BF16, kind="ExternalInput").ap()
    # [128, (chunk, g, 64)]: gather (chunk, g) covers k=2g,2g+1 for the
    # chunk's 512 nodes; position i at [i%16 (x8 partition groups), i//16]
    knn_i = nc.dram_tensor("knn_i", [P, NH * NG * COLS_H], I16, kind="ExternalInput").ap()
    # packed weights: [128, (w, kc, 256)]; order: wpT,wxf,wxd,wxv,wmf,wmd,w1,w2
    wz = nc.dram_tensor("wz", [P, 8 * CK * C], BF16, kind="ExternalInput").ap()
    bias = nc.dram_tensor("bias", [P, 10], F32, kind="ExternalInput").ap()
    outT = nc.dram_tensor("outT", [C, N], BF16, kind="ExternalOutput").ap()

    with tile.TileContext(nc) as tc:
        for _ in range(reps):
            kernel_body(tc, xT, knn_i, wz, bias, outT)
    nc.compile()
    return nc


def kernel_body(tc, xT, knn_i, wz, bias, outT):
    nc = tc.nc

    with (
        tc.tile_pool(name="const", bufs=1) as cpool,
        tc.tile_pool(name="projp", bufs=1) as projp,
        tc.tile_pool(name="gat", bufs=1) as gat,
        tc.tile_pool(name="mt", bufs=3) as mtp,
        tc.tile_pool(name="units", bufs=3) as up,
        tc.tile_pool(name="outp", bufs=3) as outp,
        tc.tile_pool(name="psA", bufs=4, space="PSUM") as psA,
        tc.tile_pool(name="psB", bufs=1, space="PSUM") as psB,
        tc.tile_pool(name="psNP", bufs=3, space="PSUM") as psNP,
        tc.tile_pool(name="dram", bufs=1, space="DRAM") as dpool,
    ):
        # ---- DMA-critical prologue: wpT, xT, chunk-0 idxs ----
        wz_sb = cpool.tile([P, 8 * CK * C], BF16)
        nc.sync.dma_start(wz_sb[:, 0:CK * C], wz[:, 0:CK * C])  # wpT first

        xt = [[None] * NQ for _ in range(CK)]
        for cc in range(NQ):
            for kc in range(CK):
                xt[kc][cc] = cpool.tile([P, QN], BF16, name=f"xt{kc}_{cc}")
                nc.sync.dma_start(xt[kc][cc][:],
                                  xT[kc * P:(kc + 1) * P, cc * QN:(cc + 1) * QN])

        kidx_all = cpool.tile([P, NH * NG * COLS_H], I16)
        HW_ = NG * COLS_H  # 512 idx cols per chunk
        nc.sync.dma_start(kidx_all[:, 0:HW_], knn_i[:, 0:HW_])

        # static schedule model for wait-until hints (ms units)
        T0 = 0.0185         # first gather transfer completes
        TP = 0.01247        # per-chunk gather window (incl. interleaved copies)

        def wslice(w_i):
            return [wz_sb[:, (w_i * CK + kc) * C:(w_i * CK + kc + 1) * C]
                    for kc in range(CK)]

        wpT_sb = wslice(0)
        wxf_sb = wslice(1)
        wxd_sb = wslice(2)
        wxv_sb = wslice(3)
        wmf_sb = wslice(4)
        wmd_sb = wslice(5)
        w1_sb = wslice(6)
        w2_sb = wslice(7)

        ident = cpool.tile([P, P], BF16)
        make_identity(nc, ident[:])
        for _ in range(24):  # PE p-state warmup while xT streams in
            wps = psNP.tile([P, 2 * C], F32, name="wps", tag="ps_np")
            nc.tensor.transpose(wps[:].bitcast(BF16)[:, 0:P], ident[:], ident[:])

        # ---- phase 1: proj -> node-major bf16 SBUF, written to DRAM per cc ----
        # proj_dram row r = p*NT + s holds node n = s*P + p (p-major permutation,
        # host permutes the knn indices to match).
        proj_sb = projp.tile([P, NT, C], BF16)
        proj_dram = dpool.tile([N, C], BF16)
        proj_dram_s = proj_dram[:].rearrange("(p s) c -> p s c", p=P)
        QT = QN // P
        for cc in range(NQ):
            for half in range(QT // 2):  # 2-stripe psum tiles
                ps = psNP.tile([P, 2 * C], F32, name="ps_np", tag="ps_np")
                for s in range(2):
                    tl = half * 2 + s
                    nc.tensor.matmul(ps[:, s * C:(s + 1) * C],
                                     lhsT=xt[0][cc][:, tl * P:(tl + 1) * P],
                                     rhs=wpT_sb[0], start=True, stop=False,
                                     skip_group_check=True)
                    nc.tensor.matmul(ps[:, s * C:(s + 1) * C],
                                     lhsT=xt[1][cc][:, tl * P:(tl + 1) * P],
                                     rhs=wpT_sb[1], start=False, stop=True,
                                     skip_group_check=True)
                dst = proj_sb[:, cc * QT + half * 2: cc * QT + half * 2 + 2, :]
                if half % 2 == 0:
                    nc.scalar.activation(dst, ps[:], AF.Copy)
                else:
                    nc.vector.tensor_copy(dst, ps[:])
            nc.sync.dma_start(proj_dram_s[:, cc * QT:(cc + 1) * QT, :],
                              proj_sb[:, cc * QT:(cc + 1) * QT, :])

        # background loads: rest of idxs, weights, bias — scheduler hint keeps
        # them from delaying the gather stream start
        bias_sb = cpool.tile([P, 10], F32)
        for ci in range(1, NH):
            with tc.tile_wait_until(T0 + (ci - 1.5) * TP if ci > 1 else 0.017):
                nc.sync.dma_start(kidx_all[:, ci * HW_:(ci + 1) * HW_],
                                  knn_i[:, ci * HW_:(ci + 1) * HW_])
        for w_i, ms in ((4, 0.021), (1, 0.022), (5, 0.0255), (2, 0.0265),
                        (3, 0.029), (6, 0.031), (7, 0.033)):
            with tc.tile_wait_until(ms):
                nc.sync.dma_start(wz_sb[:, w_i * CK * C:(w_i + 1) * CK * C],
                                  wz[:, w_i * CK * C:(w_i + 1) * CK * C])
        with tc.tile_wait_until(0.021):
            nc.sync.dma_start(bias_sb[:], bias[:])

        def kidx(i, g):
            base = (i * NG + g) * COLS_H
            return kidx_all[:, base:base + COLS_H]

        # ---- phases 2-4: software-pipelined half-quarter chunks ----
        state = {}

        def issue_gather_chain(i):
            # 8 gathers; gather g = k-pair (2g, 2g+1) x 512 nodes.
            # out position j: j//512 selects the k of the pair, j%512 the node;
            # slices [:, 0:HT, :] and [:, HT:2*HT, :] are the two k's.
            gk = [None] * NG
            for g in range(NG):
                t = gat.tile([P, 2 * HT, C], BF16, name=f"g_{i}_{g}", tag="gk",
                             bufs=16)
                with tc.tile_wait_until(max(T0 + i * TP - 0.003, 0.0)):
                    nc.gpsimd.dma_gather(
                        out_ap=t[:],
                        in_ap=proj_dram[:],
                        idxs_ap=kidx(i, g),
                        num_idxs=QN,
                        num_idxs_reg=QN,
                        elem_size=C,
                        queue_num=g % 4,
                    )
                gk[g] = t
            # fold at full tile width (free=2048) to amortize DVE op overhead:
            # acc2 = max over gathers, then fold the two k-halves at the end
            acc2 = gat.tile([P, 2 * HT, C], BF16, name=f"acc2_{i}", tag="acc2",
                            bufs=2)
            with tc.tile_wait_until(T0 + i * TP + 0.003):
                nc.vector.tensor_tensor(out=acc2[:], in0=gk[0][:], in1=gk[1][:],
                                        op=mybir.AluOpType.max)
            for g in range(2, NG):
                with tc.tile_wait_until(T0 + i * TP + (g + 1) * 0.0015):
                    nc.vector.tensor_tensor(out=acc2[:], in0=acc2[:],
                                            in1=gk[g][:],
                                            op=mybir.AluOpType.max)
                if g >= 1:
                    # PE keep-warm: tiny transpose gated on this chunk's
                    # gather g so it lands inside the PE idle window and the
                    # tensor engine never drops out of its fast p-state
                    wps = psNP.tile([P, 2 * C], F32, name="wps", tag="ps_np")
                    nc.tensor.transpose(wps[:].bitcast(BF16)[:, 0:P],
                                        gk[g][:, 0, 0:P], ident[:])
            acc = gat.tile([P, HT, C], BF16, name=f"acc{i}", tag="acc", bufs=3)
            with tc.tile_wait_until(T0 + (i + 1) * TP):
                nc.vector.tensor_tensor(out=acc[:], in0=acc2[:, 0:HT, :],
                                        in1=acc2[:, HT:2 * HT, :],
                                        op=mybir.AluOpType.max)
            return acc

        def issue_transpose(i, acc):
            # m^T via PE transpose: [node, c] -> [c, node]
            mt = mtp.tile([P, CK, HN], BF16, name="mt", tag="mt")
            with tc.tile_wait_until(T0 + (i + 1) * TP):
                for kc in range(CK):
                    pst = psB.tile([P, HN], BF16, name="pst", tag="pst")
                    for j in range(HT):
                        nc.tensor.transpose(pst[:, j * P:(j + 1) * P],
                                            acc[:, j, kc * P:(kc + 1) * P],
                                            ident[:])
                    nc.scalar.activation(mt[:, kc, :], pst[:], AF.Copy)
            state[i] = mt

        def issue_passes(i):
            mt = state.pop(i)
            q, hh = i // 2, i % 2
            xsl = slice(hh * HN, (hh + 1) * HN)
            tail = i == NH - 1
            f_sb = up.tile([P, CK, HN], BF16, name="f_sb", tag="f")
            d_sb = up.tile([P, CK, HN], BF16, name="d_sb", tag="d")
            v_sb = up.tile([P, CK, HN], BF16, name="v_sb", tag="v")
            h_sb = up.tile([P, CK, HN], BF16, name="h_sb", tag="h")
            a_sb = up.tile([P, CK, HN], BF16, name="a_sb", tag="a")

            def evac(out_sb, mc, ps, func, bcol, eng):
                b = bias_sb[:, bcol + mc:bcol + mc + 1]
                if eng == "dve":
                    nc.vector.tensor_scalar(out=out_sb[:, mc, :], in0=ps[:],
                                            scalar1=b, scalar2=None,
                                            op0=mybir.AluOpType.add)
                else:
                    nc.scalar.activation(out_sb[:, mc, :], ps[:], func,
                                         bias=b, scale=1.0)

            def mx_pass(out_sb, wm_sb, wx_sb, bcol, mc, pi, func=AF.Identity):
                ps = psA.tile([P, HN], F32, name="ps_fp", tag="ps_fp")
                first = True
                if wm_sb is not None:
                    for kc in range(CK):
                        nc.tensor.matmul(
                            ps[:], lhsT=wm_sb[kc][:, mc * P:(mc + 1) * P],
                            rhs=mt[:, kc, :], start=first and kc == 0,
                            stop=False, skip_group_check=True)
                    first = False
                for kc in range(CK):
                    nc.tensor.matmul(
                        ps[:], lhsT=wx_sb[kc][:, mc * P:(mc + 1) * P],
                        rhs=xt[kc][q][:, xsl],
                        start=first and kc == 0, stop=kc == CK - 1,
                        skip_group_check=True)
                eng = "act"
                if tail and func == AF.Identity and (mc + pi) % 2 == 1:
                    eng = "dve"
                evac(out_sb, mc, ps, func, bcol, eng)

            def hx_pass(out_sb, w_sb, in_sb, bcol, mc, pi, func=AF.Identity):
                ps = psA.tile([P, HN], F32, name="ps_fp", tag="ps_fp")
                for kc in range(CK):
                    nc.tensor.matmul(
                        ps[:], lhsT=w_sb[kc][:, mc * P:(mc + 1) * P],
                        rhs=in_sb[:, kc, :],
                        start=kc == 0, stop=kc == CK - 1,
                        skip_group_check=True)
                eng = "act"
                if tail and func == AF.Identity and (mc + pi) % 2 == 1:
                    eng = "dve"
                evac(out_sb, mc, ps, func, bcol, eng)

            tb = T0 + (i + 1) * TP
            with tc.tile_wait_until(tb + 0.001):
                for mc in range(CK):
                    mx_pass(f_sb, wmf_sb, wxf_sb, 0, mc, 0)
                    mx_pass(d_sb, wmd_sb, wxd_sb, 2, mc, 1)
                    mx_pass(v_sb, None, wxv_sb, 4, mc, 0)
            with tc.tile_wait_until(tb + 0.003):
                for mc in range(CK):
                    hx_pass(h_sb, w1_sb, f_sb, 6, mc, 1)
            with tc.tile_wait_until(tb + 0.004):
                for mc in range(CK):
                    hx_pass(a_sb, w2_sb, h_sb, 8, mc, 0, func=AF.Sigmoid)

            # out = v + a*d'
            with tc.tile_wait_until(tb + 0.0055):
                for mc in range(CK):
                    ot = outp.tile([P, HN], BF16, name="ot", tag="ot")
                    nc.vector.tensor_tensor(out=ot[:], in0=a_sb[:, mc, :],
                                            in1=d_sb[:, mc, :],
                                            op=mybir.AluOpType.mult)
                    nc.vector.tensor_tensor(out=ot[:], in0=ot[:],
                                            in1=v_sb[:, mc, :],
                                            op=mybir.AluOpType.add)
                    nc.sync.dma_start(
                        outT[mc * P:(mc + 1) * P, i * HN:(i + 1) * HN], ot[:])

        for i in range(NH):
            acc = issue_gather_chain(i)
            if i > 0:
                issue_passes(i - 1)
            issue_transpose(i, acc)
        issue_passes(NH - 1)


# ---------------- host side ----------------

def _fold(proj_W, local_W, glob_W, aff_W1, aff_b1, aff_W2, aff_b2,
          bn_local, bn_glob, bn_aff1, bn_aff2, bn_final):
    f32 = np.float32

    def bn_st(p):
        p = np.asarray(p, f32)
        g, b, m, v = p
        s = g / np.sqrt(v + EPS)
        return s.astype(f32), (b - m * s).astype(f32)

    Wp = np.asarray(proj_W, f32)
    s_l, t_l = bn_st(bn_local)
    s_g, t_g = bn_st(bn_glob)
    s_1, t_1 = bn_st(bn_aff1)
    s_2, t_2 = bn_st(bn_aff2)
    s_f, t_f = bn_st(bn_final)

    Wlp = s_l[:, None] * np.asarray(local_W, f32)
    Wgp = s_g[:, None] * np.asarray(glob_W, f32)
    Wlproj = (Wlp @ Wp).astype(f32)

    wpT = Wp.T
    wxf = (Wgp - Wlproj).T
    wxd = (-s_f[:, None] * (Wlproj + Wgp)).T
    wxv = (s_f[:, None] * Wgp).T
    wmf = Wlp.T
    wmd = (s_f[:, None] * Wlp).T
    w1 = (s_1[:, None] * np.asarray(aff_W1, f32)).T
    w2 = (s_2[:, None] * np.asarray(aff_W2, f32)).T

    ws = [wpT, wxf, wxd, wxv, wmf, wmd, w1, w2]
    wzp = np.zeros((P, 8 * CK * C), ml_dtypes.bfloat16)
    for w_i, m in enumerate(ws):
        for kc in range(CK):
            wzp[:, (w_i * CK + kc) * C:(w_i * CK + kc + 1) * C] = \
                m[kc * P:(kc + 1) * P, :].astype(ml_dtypes.bfloat16)

    tf = t_l + t_g
    td = s_f * (t_l - t_g)
    tv = s_f * t_g + t_f
    t1 = s_1 * np.asarray(aff_b1, f32) + t_1
    t2 = s_2 * np.asarray(aff_b2, f32) + t_2
    # bias[p, 2*j + mc] = coeff_j[mc*128 + p]
    bias = np.zeros((P, 10), f32)
    for j, tt in enumerate((tf, td, tv, t1, t2)):
        for mc in range(CK):
            bias[:, 2 * j + mc] = tt[mc * P:(mc + 1) * P]
    return {"wz": wzp, "bias": bias}


_NC_CACHE = {}


def _get_nc():
    if "nc" not in _NC_CACHE:
        _NC_CACHE["nc"] = build_bass(B)
    return _NC_CACHE["nc"]


def kernel(**inputs) -> np.ndarray:
    x = np.asarray(inputs["x"], np.float32)                            # [B,N,C]
    knn = np.asarray(inputs["knn"]).astype(np.int64)                   # [B,N,K]
    w = _fold(
        inputs["proj_W"], inputs["local_W"], inputs["glob_W"],
        inputs["aff_W1"], inputs["aff_b1"], inputs["aff_W2"], inputs["aff_b2"],
        inputs["bn_local"], inputs["bn_glob"], inputs["bn_aff1"],
        inputs["bn_aff2"], inputs["bn_final"],
    )

    # proj_dram row permutation: node n lives at row (n%128)*32 + n//128
    r = ((knn % P) * NT + knn // P).astype(np.int16)                   # [B,N,K]
    # gather (chunk, g): position j -> idx value r[chunk*512 + j%512, 2g + j//512]
    rr = r.reshape(B, NH, HN, NG, 2)                                   # [B,8,512,8,2]
    flat = rr.transpose(0, 1, 3, 4, 2).reshape(B, NH, NG, QN)          # [B,8,8,1024]
    wrapped = flat.reshape(B, NH, NG, COLS_H, 16).transpose(0, 4, 1, 2, 3)
    ridx = np.tile(wrapped, (1, 8, 1, 1, 1)).reshape(B, P, NH * NG * COLS_H)

    nc = _get_nc()
    in_maps = []
    for b in range(B):
        m = {"xT": np.ascontiguousarray(x[b].T).astype(ml_dtypes.bfloat16),
             "knn_i": np.ascontiguousarray(ridx[b]),
             "wz": w["wz"], "bias": w["bias"]}
        in_maps.append(m)

    res = run_bass_kernel_spmd(nc, in_maps, core_ids=list(range(B)))
    out = np.stack([res.results[b]["outT"].astype(np.float32).T
                    for b in range(B)])
    return out


if __name__ == "__main__":
    nc = build_bass(1)
    print("built OK")


# revision 12
# speedup vs baseline: 1.0759x; 1.0759x over previous
"""Trainium2 Bass kernel for the DGCNN-style message-passing block.

Math (per batch b, data-parallel over 8 cores):
    proj = x @ Wp^T
    m[i] = max_k proj[knn[i,k]]           (edge maxpool: max_k(f_j - f_i) = m_i - proj_i)
    x1 = BN_l((m - proj) @ Wl^T);  x2 = BN_g(x @ Wg^T)
    h  = BN_1((x1+x2) @ W1^T + b1); a = sigmoid(BN_2(h @ W2^T + b2))
    out = BN_f(a*x1 + (1-a)*x2)

All BNs are inference-affine and fold into the weights host-side, and proj
composes into the local branch, giving:
    f  = m@Wmf^T + x@Wxf^T + tf          (= x1+x2, feeds h)
    d' = m@Wmd^T + x@Wxd^T + td          (= s_f*(x1-x2))
    v  = x@Wxv^T + tv                    (= BN_f(x2))
    h  = f@W1'^T + t1;  a = sigmoid(h@W2'^T + t2)
    out = v + a*d'

Everything on chip is bf16 (weights, activations, gathered neighbors); PSUM
accumulates in f32.  The KNN max-gather runs per 512-node half-quarter as 8
1024-index `dma_gather` ops (the SWDGE descriptor ring caps a gather at 1024
indices), each covering a k-pair, spread over 4 SWDGE queues, sourced from a
bf16 node-major copy of proj in DRAM that is written out incrementally as
the proj matmuls complete.  A DVE max-accumulate chain reduces the 16
neighbor slices, m^T comes back via PE transposes (evacuated on the scalar
engine), and the five matmul passes run on half-width (512-node) PSUM tiles
with scalar-engine evacuation.  The pipeline is software-issued per
half-quarter (gathers+chain of chunk i before the passes of chunk i-1) so
the DVE chain tracks the gather stream, and the non-critical background
loads carry a scheduler wait-until hint so they cannot delay the gather
stream start.
"""

import numpy as np
import ml_dtypes

import concourse.bass as bass
import concourse.mybir as mybir
import concourse.tile as tile
from concourse import bacc
from concourse.bass_utils import run_bass_kernel_spmd
from concourse.masks import make_identity

F32 = mybir.dt.float32
BF16 = mybir.dt.bfloat16
I16 = mybir.dt.int16

B, N, K, C = 8, 4096, 16, 256
P = 128
NT = N // P          # 32 node stripes
CK = C // P          # 2 channel chunks
NQ = 4               # node quarters (xt granularity)
QN = N // NQ         # 1024 nodes per quarter
NH = 8               # pipeline chunks (half-quarters)
HN = N // NH         # 512 nodes per chunk
HT = HN // P         # 4 stripes per chunk
NG = K // 2          # 8 gathers per chunk (one per k-pair)
COLS_H = QN // 16    # 64 idx columns per 1024-idx gather
EPS = 1e-5

AF = mybir.ActivationFunctionType


def build_bass(n_cores: int = 8, reps: int = 1):
    nc = bacc.Bacc(
        "TRN2",
        target_bir_lowering=False,
        debug=False,
        enable_asserts=False,
        num_devices=n_cores,
        num_swdge_queues=4,
    )

    xT = nc.dram_tensor("xT", [C, N], BF16, kind="ExternalInput").ap()
    # [128, (chunk, g, 64)]: gather (chunk, g) covers k=2g,2g+1 for the
    # chunk's 512 nodes; position i at [i%16 (x8 partition groups), i//16]
    knn_i = nc.dram_tensor("knn_i", [P, NH * NG * COLS_H], I16, kind="ExternalInput").ap()
    # packed weights: [128, (w, kc, 256)]; order: wpT,wxf,wxd,wxv,wmf,wmd,w1,w2
    wz = nc.dram_tensor("wz", [P, 8 * CK * C], BF16, kind="ExternalInput").ap()
    bias = nc.dram_tensor("bias", [P, 10], F32, kind="ExternalInput").ap()
    outT = nc.dram_tensor("outT", [C, N], BF16, kind="ExternalOutput").ap()

    with tile.TileContext(nc) as tc:
        for _ in range(reps):
            kernel_body(tc, xT, knn_i, wz, bias, outT)
    nc.compile()
    return nc


def kernel_body(tc, xT, knn_i, wz, bias, outT):
    nc = tc.nc

    with (
        tc.tile_pool(name="const", bufs=1) as cpool,
        tc.tile_pool(name="projp", bufs=1) as projp,
        tc.tile_pool(name="gat", bufs=1) as gat,
        tc.tile_pool(name="mt", bufs=3) as mtp,
        tc.tile_pool(name="units", bufs=3) as up,
        tc.tile_pool(name="outp", bufs=3) as outp,
        tc.tile_pool(name="psA", bufs=4, space="PSUM") as psA,
        tc.tile_pool(name="psB", bufs=1, space="PSUM") as psB,
        tc.tile_pool(name="psNP", bufs=3, space="PSUM") as psNP,
        tc.tile_pool(name="dram", bufs=1, space="DRAM") as dpool,
    ):
        # ---- DMA-critical prologue: wpT, xT, chunk-0 idxs ----
        wz_sb = cpool.tile([P, 8 * CK * C], BF16)
        nc.sync.dma_start(wz_sb[:, 0:CK * C], wz[:, 0:CK * C])  # wpT first

        xt = [[None] * NQ for _ in range(CK)]
        for cc in range(NQ):
            for kc in range(CK):
                xt[kc][cc] = cpool.tile([P, QN], BF16, name=f"xt{kc}_{cc}")
                nc.sync.dma_start(xt[kc][cc][:],
                                  xT[kc * P:(kc + 1) * P, cc * QN:(cc + 1) * QN])

        kidx_all = cpool.tile([P, NH * NG * COLS_H], I16)
        HW_ = NG * COLS_H  # 512 idx cols per chunk
        nc.sync.dma_start(kidx_all[:, 0:HW_], knn_i[:, 0:HW_])

        # static schedule model for wait-until hints (ms units)
        T0 = 0.0185         # first gather transfer completes
        TP = 0.01247        # per-chunk gather window (incl. interleaved copies)

        def wslice(w_i):
            return [wz_sb[:, (w_i * CK + kc) * C:(w_i * CK + kc + 1) * C]
                    for kc in range(CK)]

        wpT_sb = wslice(0)
        wxf_sb = wslice(1)
        wxd_sb = wslice(2)
        wxv_sb = wslice(3)
        wmf_sb = wslice(4)
        wmd_sb = wslice(5)
        w1_sb = wslice(6)
        w2_sb = wslice(7)

        ident = cpool.tile([P, P], BF16)
        make_identity(nc, ident[:])
        for _ in range(24):  # PE p-state warmup while xT streams in
            wps = psNP.tile([P, 2 * C], F32, name="wps", tag="ps_np")
            nc.tensor.transpose(wps[:].bitcast(BF16)[:, 0:P], ident[:], ident[:])

        # ---- phase 1: proj -> node-major bf16 SBUF, written to DRAM per cc ----
        # proj_dram row r = p*NT + s holds node n = s*P + p (p-major permutation,
        # host permutes the knn indices to match).
        proj_sb = projp.tile([P, NT, C], BF16)
        proj_dram = dpool.tile([N, C], BF16)
        proj_dram_s = proj_dram[:].rearrange("(p s) c -> p s c", p=P)
        QT = QN // P
        for cc in range(NQ):
            for half in range(QT // 2):  # 2-stripe psum tiles
                ps = psNP.tile([P, 2 * C], F32, name="ps_np", tag="ps_np")
                for s in range(2):
                    tl = half * 2 + s
                    nc.tensor.matmul(ps[:, s * C:(s + 1) * C],
                                     lhsT=xt[0][cc][:, tl * P:(tl + 1) * P],
                                     rhs=wpT_sb[0], start=True, stop=False,
                                     skip_group_check=True)
                    nc.tensor.matmul(ps[:, s * C:(s + 1) * C],
                                     lhsT=xt[1][cc][:, tl * P:(tl + 1) * P],
                                     rhs=wpT_sb[1], start=False, stop=True,
                                     skip_group_check=True)
                dst = proj_sb[:, cc * QT + half * 2: cc * QT + half * 2 + 2, :]
                if half % 2 == 0:
                    nc.scalar.activation(dst, ps[:], AF.Copy)
                else:
                    nc.vector.tensor_copy(dst, ps[:])
            nc.sync.dma_start(proj_dram_s[:, cc * QT:(cc + 1) * QT, :],
                              proj_sb[:, cc * QT:(cc + 1) * QT, :])

        # background loads: rest of idxs, weights, bias — scheduler hint keeps
        # them from delaying the gather stream start
        bias_sb = cpool.tile([P, 10], F32)
        for ci in range(1, NH):
            with tc.tile_wait_until(T0 + (ci - 1.5) * TP if ci > 1 else 0.017):
                nc.sync.dma_start(kidx_all[:, ci * HW_:(ci + 1) * HW_],
                                  knn_i[:, ci * HW_:(ci + 1) * HW_])
        for w_i, ms in ((4, 0.021), (1, 0.022), (5, 0.0255), (2, 0.0265),
                        (3, 0.029), (6, 0.031), (7, 0.033)):
            with tc.tile_wait_until(ms):
                nc.sync.dma_start(wz_sb[:, w_i * CK * C:(w_i + 1) * CK * C],
                                  wz[:, w_i * CK * C:(w_i + 1) * CK * C])
        with tc.tile_wait_until(0.021):
            nc.sync.dma_start(bias_sb[:], bias[:])

        def kidx(i, g):
            base = (i * NG + g) * COLS_H
            return kidx_all[:, base:base + COLS_H]

        # ---- phases 2-4: software-pipelined half-quarter chunks ----
        state = {}

        def issue_gather_chain(i):
            # 8 gathers; gather g = k-pair (2g, 2g+1) x 512 nodes.
            # out position j: j//512 selects the k of the pair, j%512 the node;
            # slices [:, 0:HT, :] and [:, HT:2*HT, :] are the two k's.
            gk = [None] * NG
            for g in range(NG):
                t = gat.tile([P, 2 * HT, C], BF16, name=f"g_{i}_{g}", tag="gk",
                             bufs=16)
                with tc.tile_wait_until(max(T0 + i * TP - 0.003, 0.0)):
                    nc.gpsimd.dma_gather(
                        out_ap=t[:],
                        in_ap=proj_dram[:],
                        idxs_ap=kidx(i, g),
                        num_idxs=QN,
                        num_idxs_reg=QN,
                        elem_size=C,
                        queue_num=g % 4,
                    )
                gk[g] = t
            # fold at full tile width (free=2048) to amortize DVE op overhead:
            # acc2 = max over gathers, then fold the two k-halves at the end
            acc2 = gat.tile([P, 2 * HT, C], BF16, name=f"acc2_{i}", tag="acc2",
                            bufs=2)
            with tc.tile_wait_until(T0 + i * TP + 0.003):
                nc.vector.tensor_tensor(out=acc2[:], in0=gk[0][:], in1=gk[1][:],
                                        op=mybir.AluOpType.max)
            for g in range(2, NG):
                with tc.tile_wait_until(T0 + i * TP + (g + 1) * 0.0015):
                    nc.vector.tensor_tensor(out=acc2[:], in0=acc2[:],
                                            in1=gk[g][:],
                                            op=mybir.AluOpType.max)
                if g in (2, 4, 6):
                    # PE keep-warm: tiny transpose gated on this chunk's
                    # gather g so it lands inside the PE idle window and the
                    # tensor engine never drops out of its fast p-state
                    wps = psNP.tile([P, 2 * C], F32, name="wps", tag="ps_np")
                    nc.tensor.transpose(wps[:].bitcast(BF16)[:, 0:P],
                                        gk[g][:, 0, 0:P], ident[:])
            acc = gat.tile([P, HT, C], BF16, name=f"acc{i}", tag="acc", bufs=3)
            with tc.tile_wait_until(T0 + (i + 1) * TP):
                nc.vector.tensor_tensor(out=acc[:], in0=acc2[:, 0:HT, :],
                                        in1=acc2[:, HT:2 * HT, :],
                                        op=mybir.AluOpType.max)
            return acc

        def issue_transpose(i, acc):
            # m^T via PE transpose: [node, c] -> [c, node]
            mt = mtp.tile([P, CK, HN], BF16, name="mt", tag="mt")
            with tc.tile_wait_until(T0 + (i + 1) * TP):
                for kc in range(CK):
                    pst = psB.tile([P, HN], BF16, name="pst", tag="pst")
                    for j in range(HT):
                        nc.tensor.transpose(pst[:, j * P:(j + 1) * P],
                                            acc[:, j, kc * P:(kc + 1) * P],
                                            ident[:])
                    nc.scalar.activation(mt[:, kc, :], pst[:], AF.Copy)
            state[i] = mt

        def issue_passes(i):
            mt = state.pop(i)
            q, hh = i // 2, i % 2
            xsl = slice(hh * HN, (hh + 1) * HN)
            tail = i == NH - 1
            f_sb = up.tile([P, CK, HN], BF16, name="f_sb", tag="f")
            d_sb = up.tile([P, CK, HN], BF16, name="d_sb", tag="d")
            v_sb = up.tile([P, CK, HN], BF16, name="v_sb", tag="v")
            h_sb = up.tile([P, CK, HN], BF16, name="h_sb", tag="h")
            a_sb = up.tile([P, CK, HN], BF16, name="a_sb", tag="a")

            def evac(out_sb, mc, ps, func, bcol, eng):
                b = bias_sb[:, bcol + mc:bcol + mc + 1]
                if eng == "dve":
                    nc.vector.tensor_scalar(out=out_sb[:, mc, :], in0=ps[:],
                                            scalar1=b, scalar2=None,
                                            op0=mybir.AluOpType.add)
                else:
                    nc.scalar.activation(out_sb[:, mc, :], ps[:], func,
                                         bias=b, scale=1.0)

            def mx_pass(out_sb, wm_sb, wx_sb, bcol, mc, pi, func=AF.Identity):
                ps = psA.tile([P, HN], F32, name="ps_fp", tag="ps_fp")
                first = True
                if wm_sb is not None:
                    for kc in range(CK):
                        nc.tensor.matmul(
                            ps[:], lhsT=wm_sb[kc][:, mc * P:(mc + 1) * P],
                            rhs=mt[:, kc, :], start=first and kc == 0,
                            stop=False, skip_group_check=True)
                    first = False
                for kc in range(CK):
                    nc.tensor.matmul(
                        ps[:], lhsT=wx_sb[kc][:, mc * P:(mc + 1) * P],
                        rhs=xt[kc][q][:, xsl],
                        start=first and kc == 0, stop=kc == CK - 1,
                        skip_group_check=True)
                eng = "act"
                if tail and func == AF.Identity and (mc + pi) % 2 == 1:
                    eng = "dve"
                evac(out_sb, mc, ps, func, bcol, eng)

            def hx_pass(out_sb, w_sb, in_sb, bcol, mc, pi, func=AF.Identity):
                ps = psA.tile([P, HN], F32, name="ps_fp", tag="ps_fp")
                for kc in range(CK):
                    nc.tensor.matmul(
                        ps[:], lhsT=w_sb[kc][:, mc * P:(mc + 1) * P],
                        rhs=in_sb[:, kc, :],
                        start=kc == 0, stop=kc == CK - 1,
                        skip_group_check=True)
                eng = "act"
                if tail and func == AF.Identity and (mc + pi) % 2 == 1:
                    eng = "dve"
                evac(out_sb, mc, ps, func, bcol, eng)

            tb = T0 + (i + 1) * TP
            with tc.tile_wait_until(tb + 0.001):
                for mc in range(CK):
                    mx_pass(f_sb, wmf_sb, wxf_sb, 0, mc, 0)
                    mx_pass(d_sb, wmd_sb, wxd_sb, 2, mc, 1)
                    mx_pass(v_sb, None, wxv_sb, 4, mc, 0)
            with tc.tile_wait_until(tb + 0.003):
                for mc in range(CK):
                    hx_pass(h_sb, w1_sb, f_sb, 6, mc, 1)
            with tc.tile_wait_until(tb + 0.004):
                for mc in range(CK):
                    hx_pass(a_sb, w2_sb, h_sb, 8, mc, 0, func=AF.Sigmoid)

            # out = v + a*d'
            with tc.tile_wait_until(tb + 0.0055):
                for mc in range(CK):
                    ot = outp.tile([P, HN], BF16, name="ot", tag="ot")
                    nc.vector.tensor_tensor(out=ot[:], in0=a_sb[:, mc, :],
                                            in1=d_sb[:, mc, :],
                                            op=mybir.AluOpType.mult)
                    nc.vector.tensor_tensor(out=ot[:], in0=ot[:],
                                            in1=v_sb[:, mc, :],
                                            op=mybir.AluOpType.add)
                    nc.sync.dma_start(
                        outT[mc * P:(mc + 1) * P, i * HN:(i + 1) * HN], ot[:])

        for i in range(NH):
            acc = issue_gather_chain(i)
            if i > 0:
                issue_passes(i - 1)
            issue_transpose(i, acc)
        issue_passes(NH - 1)


# ---------------- host side ----------------

def _fold(proj_W, local_W, glob_W, aff_W1, aff_b1, aff_W2, aff_b2,
          bn_local, bn_glob, bn_aff1, bn_aff2, bn_final):
    f32 = np.float32

    def bn_st(p):
        p = np.asarray(p, f32)
        g, b, m, v = p
        s = g / np.sqrt(v + EPS)
        return s.astype(f32), (b - m * s).astype(f32)

    Wp = np.asarray(proj_W, f32)
    s_l, t_l = bn_st(bn_local)
    s_g, t_g = bn_st(bn_glob)
    s_1, t_1 = bn_st(bn_aff1)
    s_2, t_2 = bn_st(bn_aff2)
    s_f, t_f = bn_st(bn_final)

    Wlp = s_l[:, None] * np.asarray(local_W, f32)
    Wgp = s_g[:, None] * np.asarray(glob_W, f32)
    Wlproj = (Wlp @ Wp).astype(f32)

    wpT = Wp.T
    wxf = (Wgp - Wlproj).T
    wxd = (-s_f[:, None] * (Wlproj + Wgp)).T
    wxv = (s_f[:, None] * Wgp).T
    wmf = Wlp.T
    wmd = (s_f[:, None] * Wlp).T
    w1 = (s_1[:, None] * np.asarray(aff_W1, f32)).T
    w2 = (s_2[:, None] * np.asarray(aff_W2, f32)).T

    ws = [wpT, wxf, wxd, wxv, wmf, wmd, w1, w2]
    wzp = np.zeros((P, 8 * CK * C), ml_dtypes.bfloat16)
    for w_i, m in enumerate(ws):
        for kc in range(CK):
            wzp[:, (w_i * CK + kc) * C:(w_i * CK + kc + 1) * C] = \
                m[kc * P:(kc + 1) * P, :].astype(ml_dtypes.bfloat16)

    tf = t_l + t_g
    td = s_f * (t_l - t_g)
    tv = s_f * t_g + t_f
    t1 = s_1 * np.asarray(aff_b1, f32) + t_1
    t2 = s_2 * np.asarray(aff_b2, f32) + t_2
    # bias[p, 2*j + mc] = coeff_j[mc*128 + p]
    bias = np.zeros((P, 10), f32)
    for j, tt in enumerate((tf, td, tv, t1, t2)):
        for mc in range(CK):
            bias[:, 2 * j + mc] = tt[mc * P:(mc + 1) * P]
    return {"wz": wzp, "bias": bias}


_NC_CACHE = {}


def _get_nc():
    if "nc" not in _NC_CACHE:
        _NC_CACHE["nc"] = build_bass(B)
    return _NC_CACHE["nc"]


def kernel(**inputs) -> np.ndarray:
    x = np.asarray(inputs["x"], np.float32)                            # [B,N,C]
    knn = np.asarray(inputs["knn"]).astype(np.int64)                   # [B,N,K]
    w = _fold(
        inputs["proj_W"], inputs["local_W"], inputs["glob_W"],
        inputs["aff_W1"], inputs["aff_b1"], inputs["aff_W2"], inputs["aff_b2"],
        inputs["bn_local"], inputs["bn_glob"], inputs["bn_aff1"],
        inputs["bn_aff2"], inputs["bn_final"],
    )

    # proj_dram row permutation: node n lives at row (n%128)*32 + n//128
    r = ((knn % P) * NT + knn // P).astype(np.int16)                   # [B,N,K]
    # gather (chunk, g): position j -> idx value r[chunk*512 + j%512, 2g + j//512]
    rr = r.reshape(B, NH, HN, NG, 2)                                   # [B,8,512,8,2]
    flat = rr.transpose(0, 1, 3, 4, 2).reshape(B, NH, NG, QN)          # [B,8,8,1024]
    wrapped = flat.reshape(B, NH, NG, COLS_H, 16).transpose(0, 4, 1, 2, 3)
    ridx = np.tile(wrapped, (1, 8, 1, 1, 1)).reshape(B, P, NH * NG * COLS_H)

    nc = _get_nc()
    in_maps = []
    for b in range(B):
        m = {"xT": np.ascontiguousarray(x[b].T).astype(ml_dtypes.bfloat16),
             "knn_i": np.ascontiguousarray(ridx[b]),
             "wz": w["wz"], "bias": w["bias"]}
        in_maps.append(m)

    res = run_bass_kernel_spmd(nc, in_maps, core_ids=list(range(B)))
    out = np.stack([res.results[b]["outT"].astype(np.float32).T
                    for b in range(B)])
    return out


if __name__ == "__main__":
    nc = build_bass(1)
    print("built OK")
